# revision 58
# baseline (speedup 1.0000x reference)
"""Trainium2 Bass kernel for CompositionalMHA (moe_routing).

Math (see reference):
  For each bank b in {q,k,v}:  proj_b = sum_{j in top4(softmax(logits_b))}
      tw_j * (x @ U_j @ V_j)
  Then 16-head causal attention over the projections, then out @ out_w.T.

Host side: the top-k selection + softmax weights depend only on the tiny
logits vectors, so they are computed here in numpy; the selected U banks are
concatenated into [d, 4*64] and the tw-scaled V banks into [4*64, d_out].
All operands are cast to bf16 host-side (PSUM accumulation stays fp32; the
2e-2 rel-err budget dwarfs bf16 rounding).

Sharding (8 cores): core c = (batch b = c//2, head-half g = c%2).
Each core gets x[b] (transposed to [d,S]), the full U-cat per bank, the
head-half columns of V-cat per bank, and the matching 512 rows of out_w.T.
It computes a partial [S, d_model] output (its 8 heads' contribution through
the output projection); the host sums the two half-contributions per batch.

Device kernel works entirely in "transposed activation" layout [feat, S]:
  hT = Ucat^T @ xT           (contract d)
  qT/kT = Vw^T @ hT          (contract 4*64)    -> [512, S]
  v    = hT^T @ Vw           (per s-tile)       -> [S, 512] (natural layout)
  scoresT[k,q] = k_h @ q_h^T per head           -> exp -> causal mask
  outT[128, q] = [1 | 0*63 | v_h]^T @ probsT    (row 0 = softmax denom,
                                                 rows 64..127 = attn)
  attnT = outT[64:128] * (1/denom broadcast across partitions)
  final[s, m] = attnT^T @ w_half                (contract feature)

Scheduling notes (PE p-state ramps 0.65->1.2->2.4GHz with ~3us of
continuous execution and drops back on idle, so PE density is superlinear):
  * Startup is wire-limited (~200GB/s effective): the q/k hT banks run
    g-major -- all eight (bank, mi, sc) PSUM chains advance one matmul per
    arriving 128-row chunk of xT/u -- so the PE tracks the DMA feed instead
    of idling behind it. Chunk transfers round-robin the three DGE rings
    (sync/scalar/gpsimd); each dma_start costs its sequencer ~700ns of
    DIRECT2D issue time, so mid-kernel DMAs stay off the scalar ring (it
    dispatches the exp activations that pace attention).
  * Attention runs qc-outer (queries 0-511 fully, then 512-1023): qc0
    stages all four head-pairs first (vbank matmuls pumped between tiles as
    PE filler), then runs their PV chains (qk fc=2,3 pumped); qc1 runs the
    hp-level stage/PV software pipeline with the qc0 OUTPUT PROJECTION
    pumped between tiles, so only qc1's projection remains as a serial
    tail.
  * Softmax denominators ride row 0 of the (partition-padded) PV output;
    per-sub reciprocals land on partitions 0/32 of a zeroed staging tile
    and ONE bf16 K=33 selector matmul broadcasts both across the 128
    output partitions (sel row 0 -> partitions 0-63, row 32 -> 64-127), so
    normalization is one matmul + one [128,512] multiply with no DRAM
    round trip. The last head-pair finishes in 128-col chunks interleaved
    with the final output-projection chains. Softmax skips
    max-subtraction: scores*scale for these inputs are O(1), far from fp32
    exp overflow, and softmax normalization is scale-invariant.
  * PSUM: hT phase uses 8x1-bank chain tiles; attention uses a 2x2-bank
    staging ring plus a 4x1-bank ring shared by PV outT pairs, vbank/qk
    chains and outproj accumulators (exactly 8 banks each phase).
"""

import numpy as np
import ml_dtypes

import concourse.bacc as bacc
import concourse.mybir as mybir
import concourse.tile as tile
from concourse.bass_utils import run_bass_kernel_spmd

F32 = mybir.dt.float32
BF16 = mybir.dt.bfloat16
AF = mybir.ActivationFunctionType

P = 128
S = 1024        # sequence length
DM = 1024       # d_model
KR = 256        # top_k * r = 4 * 64
F = 512         # features per core = 8 heads * 64
NH = 8          # heads per core
HD = 64         # head dim
NG_D = DM // P  # 8
NG_R = KR // P  # 2
NG_F = F // P   # 4
NST = S // P    # 8
NSC = S // 512  # 2

TRACE = False
_cache = {}


def _emit(nc, tc, xT, us, vs, w, mask, out):
    from contextlib import ExitStack

    with ExitStack() as ctx:
        pp = ctx.enter_context(tc.tile_pool(name="persist", bufs=1))

        xT_sb = pp.tile([P, NG_D, S], BF16)
        u_sb = {b: pp.tile([P, NG_D, KR], BF16, name=f"u{b}_sb") for b in "qkv"}
        vw_sb = {b: pp.tile([P, NG_R, F], BF16, name=f"vw{b}_sb") for b in "qkv"}
        mask_sb = pp.tile([P, 2, P], BF16)
        w_sb = pp.tile([P, NG_F, DM], BF16)

        # tier 1: u banks packed host-side into [128, 8*256] (4KB/partition
        # contiguous lines -- the DMA queues bill ~64ns per descriptor line
        # regardless of size, so 512B-line chunked loads were 4x the queue
        # time) as ONE transfer each, then xT per-chunk for g-major pacing.
        rings = [nc.sync, nc.gpsimd, nc.scalar]
        nc.sync.dma_start(out=u_sb["q"][:, 0:4, :], in_=us["q"][:, 0:4 * KR])
        nc.gpsimd.dma_start(out=u_sb["k"][:, 0:4, :], in_=us["k"][:, 0:4 * KR])
        nc.scalar.dma_start(out=mask_sb, in_=mask)
        ri = [0]
        for g in range(NG_D):
            # x half-chunks round-robin all three rings for transfer-level
            # parallelism; the scalar-ring issue work here completes long
            # before the first scalar-engine compute dispatches.
            for h_ in range(2):
                rings[ri[0] % 3].dma_start(
                    out=xT_sb[:, g, h_ * 512:(h_ + 1) * 512],
                    in_=xT[g * P:(g + 1) * P, h_ * 512:(h_ + 1) * 512])
                ri[0] += 1
            if g == 2:
                nc.sync.dma_start(out=u_sb["q"][:, 4:8, :], in_=us["q"][:, 4 * KR:])
                nc.gpsimd.dma_start(out=u_sb["k"][:, 4:8, :], in_=us["k"][:, 4 * KR:])
            if g == 5:
                # qk V-banks (packed [128, 2*512]) slotted in here: they
                # must land right when the hT chains stop.
                nc.sync.dma_start(out=vw_sb["q"][:, :, :], in_=vs["q"])
                nc.gpsimd.dma_start(out=vw_sb["k"][:, :, :], in_=vs["k"])
        nc.scalar.dma_start(out=vw_sb["v"][:, :, :], in_=vs["v"])
        # tier 3: v-bank (consumed mid-qc0) and w (consumed from qc1 on).
        nc.sync.dma_start(out=u_sb["v"][:, :, :], in_=us["v"])
        for g in range(NG_F):
            nc.scalar.dma_start(out=w_sb[:, g, :], in_=w[g * P:(g + 1) * P, :])

        qT_sb = pp.tile([P, NG_F, S], BF16)
        kT_sb = pp.tile([P, NG_F, S], BF16)
        # per-head v in natural layout: column 0 = all-ones (denominator ->
        # PV row 0), columns 1-63 zero pad, columns 64-127 = v, so both the
        # denominator row (0) and the attn rows (64..127) of the PV output
        # sit at legally aligned partition bases (a DVE requirement: 64-row
        # accesses must be 64-aligned). Matmul cost is N-driven, so the
        # padded M=128 costs the same as M=65.
        VW = 128
        vS_sb = pp.tile([P, NST, NH, VW], BF16)
        nc.vector.memset(vS_sb[:, :, :, 0:1], 1.0)
        nc.vector.memset(vS_sb[:, :, :, 1:64], 0.0)
        attnT_sb = pp.tile([P, NG_F, S], BF16)
        # selector weights for the denominator broadcast: one fp32 K=33
        # matmul maps rcp2 row 0 -> output partitions 0-63 (sub0) and row 32
        # -> partitions 64-127 (sub1); rows 1-31 are zero so the zeroed
        # filler rows of rcp2 contribute nothing.
        sel_sb = pp.tile([P, P], BF16)
        nc.gpsimd.memset(sel_sb[0:64, :], 0.0)
        nc.gpsimd.memset(sel_sb[0:1, 0:HD], 1.0)
        nc.gpsimd.memset(sel_sb[32:33, HD:P], 1.0)
        # ping-pong reciprocal staging rows (written at partitions 0 and 32)
        rcp2_sb = [pp.tile([P, 512], BF16, name=f"rcp2_{i}") for i in range(2)]
        for t_ in rcp2_sb:
            nc.gpsimd.memset(t_[0:33, :], 0.0)

        hT_sb = {}
        # one shared work pool: per-tag bufs are set at tile() call sites,
        # and fewer pools shortens the end-of-kernel teardown barrier.
        spp = ctx.enter_context(tc.tile_pool(name="spp", bufs=28))
        hpool = spp
        spr = spp
        spo = spp

        # ---- Phase A: hT for q,k banks, g-major over arriving chunks ----
        with tc.tile_pool(name="php", bufs=1, space="PSUM") as php:
            h_ps = {}
            for b in "qk":
                hT_sb[b] = hpool.tile([P, NG_R, S], BF16, name=f"hT_{b}", tag="hT", bufs=3)
                for mi in range(NG_R):
                    for sc in range(NSC):
                        h_ps[(b, mi, sc)] = php.tile(
                            [P, 512], F32, name=f"h_{b}{mi}{sc}",
                            tag=f"h_{b}{mi}{sc}")
            for g in range(NG_D):
                for sc in range(NSC):
                    for mi in range(NG_R):
                        for b in "qk":
                            nc.tensor.matmul(
                                h_ps[(b, mi, sc)],
                                lhsT=u_sb[b][:, g, mi * P:(mi + 1) * P],
                                rhs=xT_sb[:, g, sc * 512:(sc + 1) * 512],
                                start=(g == 0), stop=(g == NG_D - 1))
                    if g == NG_D - 1:
                        # chain (b,mi,sc) just stopped: copy immediately so
                        # the qk projections start without a bulk-copy
                        # stall; q-bank on Scalar, k-bank on Vector so the
                        # two run in parallel (both engines idle here).
                        for mi in range(NG_R):
                            nc.scalar.copy(
                                out=hT_sb["q"][:, mi, sc * 512:(sc + 1) * 512],
                                in_=h_ps[("q", mi, sc)])
                            nc.vector.tensor_copy(
                                hT_sb["k"][:, mi, sc * 512:(sc + 1) * 512],
                                h_ps[("k", mi, sc)])

        # ---- attention + projections: 8-bank PSUM arena in three rings ----
        # (separate pools so long-lived PV accumulators never share a
        # round-robin ring with transient tiles -- the in-order PE queue
        # would deadlock on a slot held across a PV chain)
        with (
            tc.tile_pool(name="ps2", bufs=2, space="PSUM") as ps2,
            tc.tile_pool(name="pso", bufs=2, space="PSUM") as pso,
            tc.tile_pool(name="psf", bufs=2, space="PSUM") as psf,
        ):
            def t2(name):
                # 2-bank staging/qk chain tiles, ring of 2 (4 banks)
                return ps2.tile([P, 2, 512], F32, name=name, tag="bank2")

            def to(name):
                # 1-bank PV outT accumulators, ring of 2 (one hp pair alive)
                return pso.tile([P, 512], F32, name=name, tag="obank")

            def t1(name):
                # 1-bank transient chain tiles (vbank, outproj), ring of 2
                return psf.tile([P, 512], F32, name=name, tag="fbank")

            def warm(n=3):
                # dependency-free LDWEIGHTS at known PE stall points: they
                # execute instantly when the queue reaches them, keeping the
                # p-state governor's activity window alive through short
                # semaphore waits so the next real matmul runs at 2.4GHz.
                for _ in range(n):
                    nc.tensor.ldweights(weights=sel_sb)

            def gen_hT_v():
                hT_sb["v"] = hpool.tile([P, NG_R, S], BF16, name="hT_v", tag="hT", bufs=3)
                for mi in range(NG_R):
                    for sc in range(NSC):
                        v_ps = t1("vh_ps")
                        for g in range(NG_D):
                            nc.tensor.matmul(
                                v_ps,
                                lhsT=u_sb["v"][:, g, mi * P:(mi + 1) * P],
                                rhs=xT_sb[:, g, sc * 512:(sc + 1) * 512],
                                start=(g == 0), stop=(g == NG_D - 1))
                            if g % 2 == 1:
                                yield
                        nc.vector.tensor_copy(
                            hT_sb["v"][:, mi, sc * 512:(sc + 1) * 512], v_ps)
                        yield

            def gen_v():
                for st in range(NST):
                    v_ps = t1("v_ps")
                    for mi in range(NG_R):
                        nc.tensor.matmul(
                            v_ps,
                            lhsT=hT_sb["v"][:, mi, st * P:(st + 1) * P],
                            rhs=vw_sb["v"][:, mi, :],
                            start=(mi == 0), stop=(mi == NG_R - 1))
                    yield
                    nc.vector.tensor_copy(
                        vS_sb[:, st, :, 64:64 + HD],
                        v_ps.rearrange("p (h e) -> p h e", h=NH))
                    yield

            def gen_vbank():
                yield from gen_hT_v()
                yield from gen_v()

            def gen_qk(fc, on_scalar=False):
                for b in "qk":
                    dst = qT_sb if b == "q" else kT_sb
                    b_ps = t2("b_ps")
                    for sc in range(NSC):
                        for mi in range(NG_R):
                            nc.tensor.matmul(
                                b_ps[:, sc, :],
                                lhsT=vw_sb[b][:, mi, fc * P:(fc + 1) * P],
                                rhs=hT_sb[b][:, mi, sc * 512:(sc + 1) * 512],
                                start=(mi == 0), stop=(mi == NG_R - 1))
                        yield
                    for sc in range(NSC):
                        d_ = dst[:, fc, sc * 512:(sc + 1) * 512]
                        if on_scalar:
                            nc.scalar.copy(out=d_, in_=b_ps[:, sc, :])
                        else:
                            nc.vector.tensor_copy(d_, b_ps[:, sc, :])
                    yield

            def emit_qk(fc):
                for _ in gen_qk(fc, on_scalar=True):
                    pass

            def tiles_of(qc):
                return [(qc, kt) for kt in range(4 * (qc + 1))]

            def emit_stage_tile(hp, qc, kt, pT):
                rel = P * kt - 512 * qc
                q0 = max(rel, 0)
                s_ps = t2("s_ps")
                for sub in range(2):
                    po = HD * sub
                    nc.tensor.matmul(
                        s_ps[:, sub, q0:512],
                        lhsT=kT_sb[po:po + HD, hp, kt * P:(kt + 1) * P],
                        rhs=qT_sb[po:po + HD, hp, qc * 512 + q0:(qc + 1) * 512],
                        start=True, stop=True)
                t = spp.tile([P, 2, 512], BF16, name="pT", tag="pT")
                pT[(qc, kt)] = t
                nc.scalar.activation(
                    out=t[:, :, q0:512], in_=s_ps[:, :, q0:512],
                    func=AF.Exp, scale=0.125)
                if rel >= 0:
                    # causal-crossing tile: cols [q0, q0+128) need the
                    # triangular mask; cols < q0 are never read. One batched
                    # GpSimd op covers both subs (per-op overhead dominates).
                    nc.gpsimd.tensor_mul(
                        t[:, :, q0:q0 + P], t[:, :, q0:q0 + P], mask_sb)

            def emit_pv_tile(hp, qc, kt, pT, o_ps):
                n_kt = 4 * (qc + 1)
                q0 = max(P * kt - 512 * qc, 0)
                for sub in range(2):
                    h = 2 * hp + sub
                    nc.tensor.matmul(
                        o_ps[sub][0:P, q0:512],
                        lhsT=vS_sb[:, kt, h, :],
                        rhs=pT[(qc, kt)][:, sub, q0:512],
                        start=(kt == 0), stop=(kt == n_kt - 1))

            fin_i = [0]

            def finish_qc(hp, qc, o_ps, on_scalar=False):
                # extract attn rows (PV rows 64..127), then normalize: the
                # two sub reciprocals are cast to bf16 rows 0/32 of a zeroed
                # staging tile; one bf16 K=33 selector matmul broadcasts
                # sub0's to output partitions 0-63 and sub1's to 64-127; one
                # [128,512] multiply normalizes both subs in place.
                rcp2 = rcp2_sb[fin_i[0] % 2]
                fin_i[0] += 1
                for sub in range(2):
                    po = HD * sub
                    d_ = attnT_sb[po:po + HD, hp, qc * 512:(qc + 1) * 512]
                    if on_scalar:
                        nc.scalar.copy(out=d_, in_=o_ps[sub][64:64 + HD, :])
                    else:
                        nc.vector.tensor_copy(d_, o_ps[sub][64:64 + HD, :])
                for sub in range(2):
                    rcp = spr.tile([P, 512], F32, name="rcp", tag="rcp",
                                   bufs=4)
                    nc.vector.reciprocal_approx_fast(
                        out=rcp[0:1, :], in_=o_ps[sub][0:1, :])
                    cp = rcp2[32 * sub:32 * sub + 1, :]
                    if on_scalar:
                        nc.scalar.copy(out=cp, in_=rcp[0:1, :])
                    else:
                        nc.vector.tensor_copy(cp, rcp[0:1, :])
                bc_ps = t1("bc_ps")
                nc.tensor.matmul(
                    bc_ps,
                    lhsT=sel_sb[0:33, :],
                    rhs=rcp2[0:33, :],
                    start=True, stop=True)
                sl = attnT_sb[:, hp, qc * 512:(qc + 1) * 512]
                nc.vector.tensor_mul(sl, sl, bc_ps)

            def gen_outproj(qc, use_t2=False):
                # output projection for query range qc (4 s-tiles); needs
                # attnT[:, :, qc] for all four hps (normalized). use_t2
                # borrows the idle staging ring for a 4-deep accumulator
                # pipeline (tail only -- no staging runs then).
                for sti in range(4):
                    st = qc * 4 + sti
                    f2 = t2("f2_ps") if use_t2 else None
                    for mc in range(NSC):
                        f_ps = f2[:, mc, :] if use_t2 else t1("f_ps")
                        for fcc in range(NG_F):
                            nc.tensor.matmul(
                                f_ps,
                                lhsT=attnT_sb[:, fcc, st * P:(st + 1) * P],
                                rhs=w_sb[:, fcc, mc * 512:(mc + 1) * 512],
                                start=(fcc == 0), stop=(fcc == NG_F - 1))
                            if fcc % 2 == 1:
                                yield
                        # bf16 partials (host sums in f32): halves store
                        # bytes so the ring FIFO can't starve later DMAs.
                        o_sb = spo.tile([P, 512], BF16, name="o_sb", tag="o_sb", bufs=6)
                        if (st * NSC + mc) % 2 == 0:
                            nc.scalar.copy(out=o_sb, in_=f_ps)
                        else:
                            nc.vector.tensor_copy(o_sb, f_ps)
                        ring = nc.gpsimd if qc == 0 else nc.sync
                        ring.dma_start(
                            out=out[st * P:(st + 1) * P, mc * 512:(mc + 1) * 512],
                            in_=o_sb)
                        yield

            def stage(hp, qc, pT, pump, rate=3):
                for (q_, kt) in tiles_of(qc):
                    emit_stage_tile(hp, q_, kt, pT)
                    for _ in range(rate):
                        next(pump, None)

            def pv_block(hp, qc, pT, pump):
                warm()
                o_ps = [to(f"o_ps{s_}") for s_ in range(2)]
                for (q_, kt) in tiles_of(qc):
                    emit_pv_tile(hp, q_, kt, pT, o_ps)
                    next(pump, None)
                finish_qc(hp, qc, o_ps)

            def stage_and_pv(hp_next, hp, qc, pT, pT_next, pump,
                             do_finish=True):
                # interleave staging of hp_next with PV chains of hp at tile
                # granularity: the PV matmuls (probs long since ready) fill
                # the PE stalls where staging waits on the exp pipeline.
                warm()
                o_ps = [to(f"o_ps{s_}") for s_ in range(2)]
                for (q_, kt) in tiles_of(qc):
                    if hp_next is not None:
                        emit_stage_tile(hp_next, q_, kt, pT_next)
                    next(pump, None)
                    emit_pv_tile(hp, q_, kt, pT, o_ps)
                    next(pump, None)
                if do_finish:
                    finish_qc(hp, qc, o_ps, on_scalar=(hp_next is None))
                return o_ps

            def finish_chunk(hp, qc, o_ps, ci, rcp2):
                # normalize a 128-col slice of attnT[*, hp, qc] -- used for
                # the last head-pair so the final output projection can
                # start per s-tile instead of waiting for the full finish.
                c0 = ci * P
                for sub in range(2):
                    po = HD * sub
                    nc.scalar.copy(
                        out=attnT_sb[po:po + HD, hp,
                                     qc * 512 + c0:qc * 512 + c0 + P],
                        in_=o_ps[sub][64:64 + HD, c0:c0 + P])
                    rcp = spr.tile([P, 512], F32, name="rcp", tag="rcp",
                                   bufs=4)
                    nc.vector.reciprocal_approx_fast(
                        out=rcp[0:1, 0:P], in_=o_ps[sub][0:1, c0:c0 + P])
                    nc.vector.tensor_copy(
                        rcp2[32 * sub:32 * sub + 1, c0:c0 + P],
                        rcp[0:1, 0:P])
                bc_ps = t1("bc_ps")
                nc.tensor.matmul(
                    bc_ps[:, 0:P],
                    lhsT=sel_sb[0:33, :],
                    rhs=rcp2[0:33, c0:c0 + P],
                    start=True, stop=True)
                sl = attnT_sb[:, hp, qc * 512 + c0:qc * 512 + c0 + P]
                nc.vector.tensor_mul(sl, sl, bc_ps[:, 0:P])

            # ---- schedule ----
            emit_qk(0)
            emit_qk(1)
            # qc0: stage all four hps -- qk(2) pumped into stage(0), qk(3)
            # into stage(1) (each must land before its hp stages), the
            # v-bank into stages 2-3 -- then the four PV blocks.
            pT0 = {hp: {} for hp in range(4)}
            g2 = gen_qk(2)
            stage(0, 0, pT0[0], g2)
            for _ in g2:
                pass
            g3 = gen_qk(3)
            stage(1, 0, pT0[1], g3)
            for _ in g3:
                pass
            g_v = gen_vbank()
            stage(2, 0, pT0[2], g_v)
            stage(3, 0, pT0[3], g_v)
            for _ in g_v:
                pass
            # qc0 PV blocks with qc1-hp0 staging pumped between tiles: the
            # stage matmuls fill the o_ps ring handoff gaps between blocks
            # and get the qc1 exp pipeline flowing early.
            pTs = {hp: {} for hp in range(4)}

            def gen_stage_tiles(hp):
                for (q_, kt) in tiles_of(1):
                    emit_stage_tile(hp, q_, kt, pTs[hp])
                    yield
            g_s = gen_stage_tiles(0)
            for hp in range(4):
                pv_block(hp, 0, pT0[hp], g_s)
            for _ in g_s:
                pass
            # qc1: hp-level stage/PV pipeline with qc0's output projection
            # pumped into the PE gaps.
            g_op = gen_outproj(0)
            o_last = None
            for hp in range(4):
                o_last = stage_and_pv(hp + 1 if hp < 3 else None, hp, 1,
                                      pTs[hp], pTs.get(hp + 1), g_op,
                                      do_finish=(hp < 3))
            for _ in g_op:
                pass
            # tail: hp3-qc1's finish in 128-col chunks, each immediately
            # followed by the output-projection chains for that s-tile.
            rcp2 = rcp2_sb[0]
            g_t = gen_outproj(1, use_t2=True)
            for ci in range(4):
                warm()
                finish_chunk(3, 1, o_last, ci, rcp2)
                for _ in range(6):
                    next(g_t, None)
            for _ in g_t:
                pass


def _build():
    nc = bacc.Bacc("TRN2", target_bir_lowering=False, debug=False, num_devices=8)
    xT = nc.dram_tensor("xT", [DM, S], BF16, kind="ExternalInput").ap()
    us = {b: nc.dram_tensor(f"u{b}", [P, NG_D * KR], BF16, kind="ExternalInput").ap()
          for b in "qkv"}
    vs = {b: nc.dram_tensor(f"v{b}", [P, NG_R * F], BF16, kind="ExternalInput").ap()
          for b in "qkv"}
    w = nc.dram_tensor("w", [F, DM], BF16, kind="ExternalInput").ap()
    mask = nc.dram_tensor("mask", [P, 2 * P], BF16, kind="ExternalInput").ap()
    out = nc.dram_tensor("out", [S, DM], BF16, kind="ExternalOutput").ap()
    with tile.TileContext(nc) as tc:
        _emit(nc, tc, xT, us, vs, w, mask, out)
    nc.compile()
    return nc


def _tri_mask():
    # tri[rk, c] = 1.0 iff c >= rk  (keep where key index <= query index
    # within a diagonal 128x128 block)
    rk = np.arange(P)[:, None]
    c = np.arange(P)[None, :]
    m = (c >= rk).astype(ml_dtypes.bfloat16)
    return np.ascontiguousarray(np.concatenate([m, m], axis=1))


def _select_bank(U, V, logits, top_k):
    lg = np.asarray(logits, np.float32)
    e = np.exp(lg - lg.max())
    wsoft = (e / e.sum()).astype(np.float32)
    ti = np.argsort(-wsoft, kind="stable")[:top_k]
    tw = wsoft[ti]
    tw = tw / tw.sum()
    Ucat = np.concatenate([U[i] for i in ti], axis=1)          # [d, k*r]
    Vcat = np.concatenate([tw[k] * V[ti[k]] for k in range(top_k)], axis=0)
    return (np.ascontiguousarray(Ucat).astype(ml_dtypes.bfloat16),
            np.ascontiguousarray(Vcat).astype(ml_dtypes.bfloat16))


def kernel(**inputs):
    x = np.asarray(inputs["x"], np.float32)          # [4, S, d]
    out_w = np.asarray(inputs["out_w"], np.float32)  # [d, d]
    top_k = int(np.asarray(inputs["top_k"]))
    assert top_k * 64 == KR, f"kernel compiled for top_k=4, got {top_k}"
    B = x.shape[0]

    cats = {}
    for b in "qkv":
        cats[b] = _select_bank(
            np.asarray(inputs[f"{b}_U"], np.float32),
            np.asarray(inputs[f"{b}_V"], np.float32),
            inputs[f"{b}_logits"], top_k)

    if "nc" not in _cache:
        _cache["nc"] = _build()
    nc = _cache["nc"]

    mask = _tri_mask()
    wT = np.ascontiguousarray(out_w.T).astype(ml_dtypes.bfloat16)
    in_maps = []
    for c in range(8):
        b, g = c // 2, c % 2
        m = {"xT": np.ascontiguousarray(x[b].T).astype(ml_dtypes.bfloat16),
             "mask": mask,
             "w": np.ascontiguousarray(wT[g * F:(g + 1) * F, :])}
        for bank in "qkv":
            Ucat, Vcat = cats[bank]
            # pack to [128, chunks*cols]: SBUF partition p holds row
            # 128*chunk + p of the logical matrix, contiguously per chunk
            up = Ucat.reshape(NG_D, P, KR).transpose(1, 0, 2)
            m[f"u{bank}"] = np.ascontiguousarray(up.reshape(P, NG_D * KR))
            vh = np.ascontiguousarray(Vcat[:, g * F:(g + 1) * F])
            vp = vh.reshape(NG_R, P, F).transpose(1, 0, 2)
            m[f"v{bank}"] = np.ascontiguousarray(vp.reshape(P, NG_R * F))
        in_maps.append(m)

    res = run_bass_kernel_spmd(nc, in_maps, core_ids=list(range(8)), trace=TRACE)
    if TRACE:
        _cache["last_results"] = res
    parts = [np.asarray(r["out"], np.float32) for r in res.results]
    full = np.stack([parts[2 * b] + parts[2 * b + 1] for b in range(B)])
    return full.astype(np.float32)
